# revision 59
# baseline (speedup 1.0000x reference)
"""Trainium2 Bass kernel for nn_AMTCL_77867757077077 (AMTCL triplet-center loss).

Key insight: the reference's [B,B] distance matrix dist[i,j] depends on j only
through targets[j], so it has just C=100 distinct columns:

    dist[i,j] = S[i, t_j],  S[i,k] = sqrt(q[k] - 2*(x @ u.T)[i,k] + (x^2 @ v.T)[i,k])

with v = 2^w, u = v*c, q[k] = sum_d v[k,d]*c[k,d]^2.  Then
    dist_ap[i] = S[i, t_i]
    dist_an[i] = min_{k != t_i, k present} S[i,k]
    per_sample = ap + relu(cc - an),  cc = centers_dist[t_i]
so the O(B^2 D) problem collapses to two [B,C] matmuls (O(B C D)).

Sharding: data-parallel over batch rows, 8 cores x 512 rows.  centers /
centers_weights replicated.  Each core emits its partial sum/B; the host adds
the 8 scalars (the "all-reduce" of the hint, done on 8 floats host-side).

Device-side structure (everything stays in the matmul's native [class k,
row i] layout; no transposes of x needed — the host ships x/c/w pre-transposed,
which is pure layout prep; x travels as fp8e4m3, c/w as bf16 — validated to
~1e-4 relative loss error against the f32 reference):
    q[k]      = -diag(cd_chain)            (free byproduct of the cd matmuls)
    S^T       = sqrt(PSUM + (q+pen)[k])    (ACT, per-partition bias)
    ohT[k,i]  = (t_bcast - k == 0)         (one DVE op)
    sum_i ap  = free accum_out of w1x = ohT*S^T        (DVE)
    cc_row    = cd^T @ ohT                 (TensorE), flipped to columns by
                tiny K=1 matmuls
    an        = row-min of transposed (S^T + 1e9*ohT) tiles (TensorE+DVE)
    loss_part = (sum ap + sum relu(cc-an)) / B via ones-matmul
"""

import math
import numpy as np

NUM_CORES = 8
B = 4096
D = 384
C = 100
BL = B // NUM_CORES  # 512 rows per core
P = 128
NT = BL // P         # 4 row tiles per core
KD = D // P          # 3 contraction chunks

_CACHE = {}


def _build_nc():
    import concourse.bass as bass
    import concourse.bacc as bacc
    import concourse.tile as tile
    from concourse import mybir
    from concourse.masks import make_identity
    from contextlib import ExitStack

    f32 = mybir.dt.float32
    bf16 = mybir.dt.bfloat16
    LN2 = float(math.log(2.0))

    nc = bacc.Bacc(
        "TRN2",
        target_bir_lowering=False,
        debug=False,
        enable_asserts=False,
        num_devices=NUM_CORES,
    )

    xt_ext = nc.dram_tensor("xt", [D, BL], mybir.dt.float8e4, kind="ExternalInput").ap()
    t_ext = nc.dram_tensor("t", [BL], bf16, kind="ExternalInput").ap()
    cw_ext = nc.dram_tensor("cw", [D, 2 * C], bf16, kind="ExternalInput").ap()
    pen_ext = nc.dram_tensor("pen", [C, 1], f32, kind="ExternalInput").ap()
    out_ext = nc.dram_tensor("out", [1, 1], f32, kind="ExternalOutput").ap()

    with tile.TileContext(nc) as tc, ExitStack() as ctx:
        singles = ctx.enter_context(tc.tile_pool(name="singles", bufs=1))
        ps_big = ctx.enter_context(tc.tile_pool(name="psbig", bufs=1, space="PSUM"))
        ps_tr = ctx.enter_context(tc.tile_pool(name="pstr", bufs=2, space="PSUM"))
        ps_misc = ctx.enter_context(tc.tile_pool(name="psmisc", bufs=1, space="PSUM"))
        ps_g = ctx.enter_context(tc.tile_pool(name="psg", bufs=2, space="PSUM"))

        # ---- input DMAs first, spread across both HWDGE queues (SP + ACT);
        # wt leads: the Exp -> center-prep chain gates everything
        cwT = singles.tile([P, KD, 2 * C], bf16)  # [c^T | w^T] packed in one DMA
        nc.sync.dma_start(cwT, cw_ext.rearrange("(k p) c -> p k c", p=P))
        ctT = cwT[:, :, 0:C]
        wtT = cwT[:, :, C : 2 * C]
        xT = singles.tile([P, KD, BL], mybir.dt.float8e4)  # x^T (fp8, host-transposed)
        xt_r = xt_ext.rearrange("(k p) i -> p k i", p=P)
        nc.sync.dma_start(xT[:, 0:2, :], xt_r[:, 0:2, :])
        nc.scalar.dma_start(xT[:, 2:, :], xt_r[:, 2:, :])
        t_row = singles.tile([1, BL], bf16)     # targets (<100, exact in bf16)
        nc.gpsimd.dma_start(t_row, t_ext.rearrange("(a i) -> a i", a=1))
        pen_sb = singles.tile([C, 1], f32)      # absent-class d2 penalty
        nc.gpsimd.dma_start(pen_sb, pen_ext)

        # ---- constants
        ident_bf = singles.tile([P, P], bf16)
        make_identity(nc, ident_bf)
        iota_col = singles.tile([P, 1], f32)   # value = partition index
        nc.gpsimd.iota(
            iota_col,
            pattern=[[0, 1]],
            base=0,
            channel_multiplier=1,
            allow_small_or_imprecise_dtypes=True,
        )
        invB_col = singles.tile([P, 1], f32)
        nc.vector.memset(invB_col, 1.0 / B)
        one11_bf = singles.tile([1, 1], bf16)
        nc.vector.memset(one11_bf, 1.0)
        ones_1p_bf = singles.tile([1, P], bf16)
        nc.vector.memset(ones_1p_bf, 1.0)

        # t broadcast to all partitions via K=1 ones-matmul (PSUM, read by DVE)
        tbc_ps = ps_g.tile([P, BL], f32, tag="g")
        nc.tensor.matmul(tbc_ps, lhsT=ones_1p_bf, rhs=t_row)

        # ---- center-side prep (all in transposed [d, class] layout)
        vT = singles.tile([P, KD, C], bf16)     # 2^w  = exp(ln2 * w)
        i_exp = nc.scalar.activation(
            vT, wtT, mybir.ActivationFunctionType.Exp, scale=LN2
        )
        # dummy sqrt pinned right after the Exp: pulls the Sqrt ACT-table load
        # off the critical path (it otherwise happens lazily before the first
        # real sqrt); the dep edge stops the scheduler hoisting it before Exp,
        # which would ping-pong the table slots
        sqrt_warm = singles.tile([1, 1], f32)
        i_sqw = nc.scalar.activation(
            sqrt_warm, invB_col[0:1, :], mybir.ActivationFunctionType.Sqrt
        )
        tile.add_dep_helper(i_sqw.ins, i_exp.ins, sync=False,
                            reason="order sqrt-table prefetch after Exp")
        ct_bf = ctT
        c2T = singles.tile([P, KD, C], bf16)    # c^2
        nc.vector.tensor_mul(c2T, ct_bf, ct_bf)
        uT2 = singles.tile([P, KD, C], bf16)    # -2 * v * c
        i_ut2 = nc.vector.scalar_tensor_tensor(
            out=uT2,
            in0=vT,
            scalar=-2.0,
            in1=ct_bf,
            op0=mybir.AluOpType.mult,
            op1=mybir.AluOpType.mult,
        )

        # ohT[k,i] = ((t_bcast[k,i] - k) == 0), exact 0/1 in bf16
        # (explicit dep edges keep the in-order DVE queue from head-of-line
        # blocking uT2 -- which gates the PE chains -- behind x^2 ops that
        # wait on the big x DMA)
        ohT = singles.tile([P, BL], bf16)
        i_oht = nc.vector.tensor_scalar(
            out=ohT, in0=tbc_ps, scalar1=iota_col[:, :], scalar2=0.0,
            op0=mybir.AluOpType.subtract, op1=mybir.AluOpType.is_equal,
        )
        tile.add_dep_helper(i_oht.ins,
                            i_ut2.ins,
                            sync=False, reason="order uT2 before ohT on DVE")
        # x^2 (needed by the second half of the main chain only); k=2 arrives
        # first (scalar-queue DMA chunk)
        x2T = singles.tile([P, KD, BL], bf16)
        prev = i_oht
        for k in (2, 0, 1):
            i_sq = nc.vector.tensor_mul(x2T[:, k, :], xT[:, k, :], xT[:, k, :])
            tile.add_dep_helper(i_sq.ins,
                                prev.ins,
                                sync=False, reason="DVE queue order")
            prev = i_sq

        # ---- closest-center distance cd[k] (100x100 matmul path)
        # cd_ps[i,j] = sum_d v[i,d]c[j,d]^2 - 2 sum_d u[i,d]c[j,d]; diag = -q[i]
        cd_ps = ps_big.tile([C, C], f32)
        for k in range(KD):
            nc.tensor.matmul(
                cd_ps, lhsT=vT[:, k, :], rhs=c2T[:, k, :],
                start=(k == 0), stop=False,
            )
        for k in range(KD):
            nc.tensor.matmul(
                cd_ps, lhsT=uT2[:, k, :], rhs=ct_bf[:, k, :],
                start=False, stop=(k == KD - 1),
            )
        # q[k] = -diag(cd_ps): mask with identity, row-sum, negate
        qdiag = singles.tile([C, C], f32)
        nc.vector.tensor_mul(qdiag, cd_ps, ident_bf[:C, :C])
        q_col = singles.tile([C, 1], f32)
        nc.vector.tensor_reduce(
            q_col[:C, :], qdiag, axis=mybir.AxisListType.X,
            op=mybir.AluOpType.add, negate=True,
        )
        qpen_col = singles.tile([C, 1], f32)
        nc.vector.tensor_add(qpen_col, q_col, pen_sb)
        # d2c = max(cd_ps + q[i], 0) then sqrt; diagonal -> 1e9; row-min
        e_sb = singles.tile([C, C], f32)
        nc.vector.tensor_scalar(
            out=e_sb, in0=cd_ps, scalar1=q_col[:C, :], scalar2=0.0,
            op0=mybir.AluOpType.add, op1=mybir.AluOpType.max,
        )
        dd_sb = singles.tile([C, C], f32)
        nc.scalar.activation(dd_sb, e_sb, mybir.ActivationFunctionType.Sqrt)
        nc.gpsimd.affine_select(
            out=dd_sb, in_=dd_sb,
            compare_op=mybir.AluOpType.not_equal,
            fill=1e9, base=0, pattern=[[-1, C]], channel_multiplier=1,
        )
        cd_colP = singles.tile([P, 1], f32)
        nc.vector.memset(cd_colP, 0.0)
        nc.vector.tensor_reduce(
            cd_colP[:C, :], dd_sb, axis=mybir.AxisListType.X, op=mybir.AluOpType.min
        )
        cd_col_bf = singles.tile([P, 1], bf16)
        nc.vector.tensor_copy(cd_col_bf, cd_colP)

        # ---- main matmul chain: S^T[k_class, i] partial d2 in PSUM
        # (xT terms first: they don't wait for the x^2 squares)
        s_ps = ps_big.tile([C, BL], f32)
        for j, k in enumerate((2, 0, 1)):
            nc.tensor.matmul(
                s_ps, lhsT=uT2[:, k, :], rhs=xT[:, k, :],
                start=(j == 0), stop=False,
            )
        for j, k in enumerate((2, 0, 1)):
            nc.tensor.matmul(
                s_ps, lhsT=vT[:, k, :], rhs=x2T[:, k, :],
                start=False, stop=(j == KD - 1),
            )
        # S^T = sqrt(d2 + q + pen), rows 100..127 zeroed
        st_sb = singles.tile([P, BL], f32)
        nc.gpsimd.memset(st_sb, 0.0)
        nc.scalar.activation(
            st_sb[:C, :], s_ps, mybir.ActivationFunctionType.Sqrt, bias=qpen_col[:, :]
        )

        # sbig = S^T + 1e9*ohT per column tile; transpose (PE) + row-min = an
        sbig = singles.tile([P, BL], bf16)
        mnc_bf = singles.tile([P, NT], bf16)
        for t in range(NT):
            sl = slice(t * P, (t + 1) * P)
            nc.vector.scalar_tensor_tensor(
                out=sbig[:, sl], in0=ohT[:, sl], scalar=1e9, in1=st_sb[:, sl],
                op0=mybir.AluOpType.mult, op1=mybir.AluOpType.add,
            )
            st_ps = ps_tr.tile([P, P], bf16)
            nc.tensor.transpose(st_ps, sbig[:, sl], ident_bf)
            nc.vector.tensor_reduce(
                mnc_bf[:, t : t + 1], st_ps[:, :C], axis=mybir.AxisListType.X,
                op=mybir.AluOpType.min,
            )

        # w1x = ohT * S^T; its free-axis accumulator already gives
        # ap_part[k] = sum_i S[i,t_i]*[t_i==k]  (summed later by the final mm)
        w1x = singles.tile([P, BL], bf16)
        ap_part = singles.tile([C, 1], f32)
        nc.vector.scalar_tensor_tensor(
            out=w1x[:C, :], in0=st_sb[:C, :], scalar=1.0, in1=ohT[:C, :],
            op0=mybir.AluOpType.mult, op1=mybir.AluOpType.mult,
            accum_out=ap_part,
        )
        del w1x  # only the accumulator is used

        # cc_row[i] = cd[t_i], flipped into [128, NT] columns (tiny K=1 matmuls)
        cc_ps = ps_g.tile([1, BL], f32, tag="g")
        nc.tensor.matmul(cc_ps, lhsT=cd_col_bf, rhs=ohT)
        cc_row = singles.tile([1, BL], bf16)
        nc.scalar.copy(cc_row, cc_ps)
        cc_col = ps_g.tile([P, NT], f32, tag="g")
        for t in range(NT):
            nc.tensor.matmul(
                cc_col[:, t : t + 1],
                lhsT=cc_row[:, t * P : (t + 1) * P],
                rhs=one11_bf,
            )

        # sum relu(cc - an) + sum ap, then / B via ones-matmul
        dcol = singles.tile([P, NT], f32)
        nc.vector.tensor_sub(dcol, cc_col, mnc_bf)
        junkc = singles.tile([P, NT], f32)
        relu_part = singles.tile([P, 1], f32)
        nc.vector.tensor_scalar(
            out=junkc, in0=dcol, scalar1=0.0, scalar2=None,
            op0=mybir.AluOpType.max, op1=mybir.AluOpType.add,
            accum_out=relu_part,
        )
        fin_ps = ps_misc.tile([1, 1], f32, tag="misc")
        nc.tensor.matmul(fin_ps, lhsT=invB_col, rhs=relu_part, start=True, stop=False)
        nc.tensor.matmul(
            fin_ps, lhsT=invB_col[:C, :], rhs=ap_part, start=False, stop=True
        )
        out_sb = singles.tile([1, 1], f32)
        nc.vector.tensor_copy(out_sb, fin_ps)
        nc.sync.dma_start(out_ext, out_sb)

    nc.compile()
    return nc


def _get_nc():
    if "nc" not in _CACHE:
        _CACHE["nc"] = _build_nc()
    return _CACHE["nc"]


def make_in_maps(inputs, targets, centers, centers_weights):
    import ml_dtypes

    x = np.asarray(inputs, np.float32)
    t = np.asarray(targets).astype(np.int64)
    c = np.asarray(centers, np.float32)
    w = np.asarray(centers_weights, np.float32)
    assert x.shape == (B, D) and c.shape == (C, D) and w.shape == (C, D)

    xt_all = x.astype(ml_dtypes.float8_e4m3).T               # [D, B] fp8
    cw = np.ascontiguousarray(
        np.concatenate([c.T, w.T], axis=1).astype(ml_dtypes.bfloat16)
    )  # [D, 2C] bf16
    t_bf = t.astype(ml_dtypes.bfloat16)     # targets < 100: exact in bf16
    present = np.bincount(t, minlength=C) > 0
    pen = np.where(present, 0.0, 1e12).astype(np.float32).reshape(C, 1)

    in_maps = []
    for i in range(NUM_CORES):
        sl = slice(i * BL, (i + 1) * BL)
        in_maps.append(
            {
                "xt": np.ascontiguousarray(xt_all[:, sl]),
                "t": np.ascontiguousarray(t_bf[sl]),
                "cw": cw,
                "pen": pen,
            }
        )
    return in_maps


def run(inputs, targets, centers, centers_weights, trace=False):
    """Build+run the SPMD kernel; returns (loss_scalar, BassKernelResults)."""
    from concourse import bass_utils

    nc = _get_nc()
    in_maps = make_in_maps(inputs, targets, centers, centers_weights)
    res = None
    for attempt in range(3):
        try:
            res = bass_utils.run_bass_kernel_spmd(
                nc, in_maps, core_ids=list(range(NUM_CORES)), trace=trace
            )
            break
        except Exception:
            # A previously-crashed session can leave the device in a transient
            # "unrecoverable" state that clears on the next attempt.
            if attempt == 2:
                raise
    loss = np.float32(0.0)
    for r in res.results:
        loss += np.float32(r["out"][0, 0])
    return np.array(loss, dtype=np.float32), res


def kernel(inputs, targets, epoch_number=None, centers=None, centers_weights=None):
    loss, _ = run(inputs, targets, centers, centers_weights, trace=False)
    return loss
